# revision 5
# baseline (speedup 1.0000x reference)
"""Trainium2 Bass kernel for causal self-attention with RoPE.

Problem: B=2, T=2048, C=1024, H=16 heads, hd=64, fp32, causal, rotary embeddings.

Sharding: 8 cores = 2 batches x 4 head-groups. Core c handles batch c//4 and
heads [4*(c%4), 4*(c%4)+4). Each core computes its heads' Q/K/V projections,
RoPE, causal attention, and a partial output projection over its 256 input
channels; the host sums the 4 partial (fp16) projections per batch and adds
the output bias.

v3 design notes:
  - Attention tiled over FOUR 512-col t-windows, interleaved with projection
    work so ACT (exp, ~86us/core floor) starts early and the PE stays dense
    (HAM keeps the 2.4 GHz clock).
  - Both heads of a pair share one [128,1024] scores PSUM tile -> ONE wide
    ACTIVATE per s-tile covers both heads.
  - att@V + denominator matmuls pack the two heads into PE column groups;
    accumulation uses memset-zeroed PSUM with start=False (no accumulation
    groups), so the interleaved chains in one bank are legal.
  - In-loop emission lags att@V one s-tile behind exp so the PE FIFO never
    blocks on the ACT engine.
  - Inputs arrive in 14 large DMAs (descriptor issue on the sync queue costs
    ~0.6us each); rope's 32-row swap uses DVE copies, not DMA.
  - fp16 output partials, one wide DMA per 128-row t-chunk.
"""

import os
import time
from contextlib import ExitStack

import ml_dtypes
import numpy as np

import concourse.bass as bass
import concourse.tile as tile
from concourse import bacc, library_config, mybir
from concourse.bass_utils import run_bass_kernel_spmd

F32 = mybir.dt.float32
F16 = mybir.dt.float16
BF16 = mybir.dt.bfloat16

T = 2048
C = 1024
HD = 64
NCORES = 8
NEG = -1e10
NW = 4            # t-windows of 512
WW = 512          # window width

AF = mybir.ActivationFunctionType
ALU = mybir.AluOpType

LAST_EXEC_NS = None
LAST_RESULTS = None


def build_nc():
    nc = bacc.Bacc("TRN2", target_bir_lowering=False, debug=False)

    x3 = nc.dram_tensor("x3", [128, 8, T], BF16, kind="ExternalInput").ap()
    wq3 = nc.dram_tensor("wq3", [128, 8, 256], BF16, kind="ExternalInput").ap()
    wk3 = nc.dram_tensor("wk3", [128, 8, 256], BF16, kind="ExternalInput").ap()
    wv3 = nc.dram_tensor("wv3", [128, 9, 260], BF16, kind="ExternalInput").ap()
    wp3 = nc.dram_tensor("wp3", [128, 2, C], BF16, kind="ExternalInput").ap()
    bqk = nc.dram_tensor("bqk", [128, 4], F32, kind="ExternalInput").ap()
    cc_d = nc.dram_tensor("cc", [128, T], F32, kind="ExternalInput").ap()
    ss_d = nc.dram_tensor("ss", [128, T], F32, kind="ExternalInput").ap()
    tri_d = nc.dram_tensor("tri", [128, 128], F32, kind="ExternalInput").ap()
    out_d = nc.dram_tensor("out", [T, C], F16, kind="ExternalOutput").ap()

    with tile.TileContext(nc) as tc, ExitStack() as ctx:
        consts = ctx.enter_context(tc.tile_pool(name="consts", bufs=1))

        # ---- persistent SBUF tiles ----
        cc_sb = consts.tile([128, T], F32)
        ss_sb = consts.tile([128, T], F32)
        tri_sb = consts.tile([128, 128], F32)
        bqk_sb = consts.tile([128, 4], F32)
        x1 = consts.tile([1, T], BF16)          # ones row (V bias matmul)
        wsrc = consts.tile([128, 512], BF16)    # warmup matmul source

        xa = consts.tile([128, 8 * T], BF16, name="xa")
        xv = xa.rearrange("p (j t) -> p j t", t=T)
        wqa = consts.tile([128, 8 * 256], BF16, name="wqa")
        wqv = wqa.rearrange("p (j c) -> p j c", c=256)
        wka = consts.tile([128, 8 * 256], BF16, name="wka")
        wkv = wka.rearrange("p (j c) -> p j c", c=256)
        wva = consts.tile([128, 9 * 260], BF16, name="wva")
        wvv = wva.rearrange("p (j c) -> p j c", c=260)
        wpa = consts.tile([128, 2 * C], BF16, name="wpa")
        wpv = wpa.rearrange("p (q c) -> p q c", c=C)

        # rotated Q^T / K^T, one [128, T] tile per head pair:
        # 0,1 = Q pair0/1; 2,3 = K pair0/1. rows [hA(64) | hB(64)].
        qkt = [consts.tile([128, T], BF16, name=f"qkt{i}") for i in range(4)]
        # V' per s-tile: [128 s, 256] = 4 heads x 64 dims
        vp = [consts.tile([128, 260], BF16, name=f"vp{i}") for i in range(16)]
        # scaled attention outputs (d-major), one [128, T] per pair
        usc = [consts.tile([128, T], BF16, name=f"usc{p}") for p in range(2)]

        # ---- PSUM pools (8 banks total) ----
        sp = ctx.enter_context(
            tc.tile_pool(name="sp", bufs=2, space="PSUM"))      # 4 banks
        yp = ctx.enter_context(
            tc.tile_pool(name="yp", bufs=3, space="PSUM"))      # 3 banks
        qp = ctx.enter_context(
            tc.tile_pool(name="qp", bufs=1, space="PSUM"))      # 1 bank

        # ---- SBUF working pools ----
        etp = ctx.enter_context(tc.tile_pool(name="etp", bufs=6))
        rp = ctx.enter_context(tc.tile_pool(name="rp", bufs=2))
        dvp = ctx.enter_context(tc.tile_pool(name="dvp", bufs=2))
        ost = ctx.enter_context(tc.tile_pool(name="ost", bufs=4))

        # ================= t=0: warmup + setup =================
        nc.vector.memset(wsrc[:], 0.0)
        warm = sp.tile([128, 1024], F32, tag="s", name="warm")
        for r in range(14):
            nc.tensor.matmul(warm[:, 0:512], wsrc[:, 0:128], wsrc[:],
                             start=True, stop=True)
        nc.vector.memset(x1[:], 1.0)
        # preload the exp ACT table set early (table load ~2.7us)
        dummy_et = consts.tile([1, 128], BF16, name="dummy_et")
        nc.scalar.activation(dummy_et[:], wsrc[0:1, 0:128], AF.Exp, scale=0.125)
        nc.gpsimd.load_library(library_config.attn)

        # ================= DMA schedule (priority = order) =================
        def wsl(w):
            return slice(WW * w, WW * (w + 1))

        nc.sync.dma_start(wqa[:], wq3[:, :, :])
        nc.sync.dma_start(xv[:, :, wsl(0)], x3[:, :, wsl(0)])
        nc.sync.dma_start(bqk_sb[:], bqk[:])
        nc.sync.dma_start(cc_sb[:, 0:1024], cc_d[:, 0:1024])
        nc.sync.dma_start(ss_sb[:, 0:1024], ss_d[:, 0:1024])
        nc.sync.dma_start(wka[:], wk3[:, :, :])
        nc.sync.dma_start(tri_sb[:], tri_d[:])
        nc.sync.dma_start(wva[:], wv3[:, :, :])
        nc.sync.dma_start(xv[:, :, wsl(1)], x3[:, :, wsl(1)])
        nc.sync.dma_start(xv[:, :, wsl(2)], x3[:, :, wsl(2)])
        nc.sync.dma_start(cc_sb[:, 1024:2048], cc_d[:, 1024:2048])
        nc.sync.dma_start(ss_sb[:, 1024:2048], ss_d[:, 1024:2048])
        nc.sync.dma_start(xv[:, :, wsl(3)], x3[:, :, wsl(3)])
        nc.sync.dma_start(wpa[:], wp3[:, :, :])

        # ================= building blocks =================
        def qk_chunk(t_idx, w):
            """Project + rope one 512-col block of qkt[t_idx]."""
            wv_ = wqv if t_idx < 2 else wkv
            pr = t_idx % 2
            bcol = (2 if t_idx >= 2 else 0) + pr
            bias = bqk_sb[:, bcol:bcol + 1]
            ps = qp.tile([128, WW], F32, tag="qp", name=f"qk{t_idx}_{w}")
            for j in range(8):
                nc.tensor.matmul(
                    ps[:], wv_[:, j, 128 * pr:128 * (pr + 1)],
                    xv[:, j, wsl(w)], start=(j == 0), stop=(j == 7))
            p1 = rp.tile([128, WW], F32, tag="p1")
            p2 = rp.tile([128, WW], F32, tag="p2")
            p2s = rp.tile([128, WW], F32, tag="p2s")
            nc.vector.scalar_tensor_tensor(
                out=p1[:], in0=ps[:], scalar=bias,
                in1=cc_sb[:, wsl(w)], op0=ALU.add, op1=ALU.mult)
            nc.vector.scalar_tensor_tensor(
                out=p2[:], in0=ps[:], scalar=bias,
                in1=ss_sb[:, wsl(w)], op0=ALU.add, op1=ALU.mult)
            for r in range(4):
                src = slice(32 * (r ^ 1), 32 * (r ^ 1) + 32)
                dst = slice(32 * r, 32 * r + 32)
                nc.vector.tensor_copy(p2s[dst, :], p2[src, :])
            nc.vector.tensor_add(qkt[t_idx][:, wsl(w)], p1[:], p2s[:])

        def v_chunk(tch):
            """V' for s-tile tch: [128 s, 256] = 4 heads x 64 dims."""
            vr = qp.tile([128, 260], F32, tag="qp", name=f"vr{tch}")
            tsl = slice(128 * tch, 128 * (tch + 1))
            for j in range(8):
                nc.tensor.matmul(vr[:], xv[:, j, tsl], wvv[:, j, :],
                                 start=(j == 0), stop=False)
            nc.tensor.matmul(vr[:], x1[:, tsl], wvv[0:1, 8, :],
                             start=False, stop=True)
            nc.vector.tensor_copy(vp[tch][:], vr[:])

        def attn_unit(p, w, u):
            """Attention for head-pair p over t-window w (s-tiles 0..4w+3)."""
            qt, kt = qkt[p], qkt[2 + p]
            ni = 4 * w + 4
            # per-head [65, WW] accumulators: rows 0:64 = y, row 64 = z
            # (ones column in V'), one PSUM bank each.
            yz = [yp.tile([65, WW], F32, tag="y", name=f"yz{p}_{w}_{hs}")
                  for hs in range(2)]
            ets = {}

            def scores_exp(i):
                lo = max(0, 128 * i - WW * w)
                sc = sp.tile([128, 1024], F32, tag="s", name=f"s{p}_{w}_{i}")
                # scores: A (rows 0:64) into cols lo:512, B (64:128) into
                # cols 512:1024 (untrimmed so the exp region is contiguous)
                nc.tensor.matmul(
                    sc[:, lo:WW],
                    kt[0:64, 128 * i:128 * (i + 1)],
                    qt[0:64, WW * w + lo:WW * (w + 1)],
                    start=True, stop=True)
                nc.tensor.matmul(
                    sc[:, WW:2 * WW],
                    kt[64:128, 128 * i:128 * (i + 1)],
                    qt[64:128, wsl(w)],
                    start=True, stop=True)
                if i >= 4 * w:  # diagonal tile: causal mask both heads
                    nc.vector.tensor_add(sc[:, lo:lo + 128],
                                         sc[:, lo:lo + 128], tri_sb[:])
                    nc.vector.tensor_add(sc[:, WW + lo:WW + lo + 128],
                                         sc[:, WW + lo:WW + lo + 128],
                                         tri_sb[:])
                et = etp.tile([128, 1024], BF16, tag="e", name=f"e{p}_{w}_{i}")
                nc.scalar.activation(et[:, lo:], sc[:, lo:], AF.Exp,
                                     scale=0.125)
                ets[i] = (et, lo)

            def yz_mm(i):
                et, lo = ets.pop(i)
                st, sp_ = (i == 0), (i == ni - 1)
                nc.tensor.matmul(yz[0][:, lo:],
                                 vp[i][:, 130 * p:130 * p + 65],
                                 et[:, lo:WW], start=st, stop=sp_)
                nc.tensor.matmul(yz[1][:, lo:],
                                 vp[i][:, 130 * p + 65:130 * p + 130],
                                 et[:, WW + lo:2 * WW], start=st, stop=sp_)

            # software pipeline: att@V lags exp by one s-tile so the PE FIFO
            # always has ready work while ACT computes the current exp.
            for i in range(ni):
                scores_exp(i)
                if i > 0:
                    yz_mm(i - 1)
            yz_mm(ni - 1)

            # normalize: usc[p][64hs:, w] = y * (1/z) broadcast per head
            for hs in range(2):
                zrow = dvp.tile([1, WW], F32, tag="zrow", name=f"zr{p}_{w}{hs}")
                rzr = dvp.tile([1, WW], F32, tag="rzr", name=f"rz{p}_{w}{hs}")
                rzb = dvp.tile([64, WW], F32, tag="rzb", name=f"rzb{p}_{w}{hs}")
                nc.vector.tensor_copy(zrow[:], yz[hs][64:65, :])
                nc.vector.reciprocal_approx_fast(rzr[:], zrow[:])
                nc.gpsimd.partition_broadcast(rzb[:], rzr[:])
                nc.vector.tensor_mul(usc[p][64 * hs:64 * (hs + 1), wsl(w)],
                                     yz[hs][0:64, :], rzb[:])

        def oproj_chunk(c, wide):
            """Output projection for t-chunk c (128 t-cols)."""
            tsl = slice(128 * c, 128 * (c + 1))
            st = ost.tile([128, C], F16, tag="o", name=f"os{c}")
            if not wide:
                for cg in range(2):
                    csl = slice(512 * cg, 512 * (cg + 1))
                    ps = qp.tile([128, 512], F32, tag="qp", name=f"op{c}_{cg}")
                    for pq in range(2):
                        nc.tensor.matmul(ps[:], usc[pq][:, tsl],
                                         wpv[:, pq, csl],
                                         start=(pq == 0), stop=(pq == 1))
                    nc.vector.tensor_copy(st[:, csl], ps[:])
            else:
                ps = sp.tile([128, 1024], F32, tag="s", name=f"opw{c}")
                for cg in range(2):
                    csl = slice(512 * cg, 512 * (cg + 1))
                    for pq in range(2):
                        nc.tensor.matmul(ps[:, csl], usc[pq][:, tsl],
                                         wpv[:, pq, csl],
                                         start=(pq == 0), stop=(pq == 1))
                nc.vector.tensor_copy(st[:], ps[:])
            nc.sync.dma_start(out_d[tsl, :], st[:])

        # ================= emission schedule =================
        u = 0
        for w in range(NW):
            qk_chunk(0, w)          # Q pair0 block w
            qk_chunk(2, w)          # K pair0 block w
            for tch in range(4 * w, 4 * w + 4):
                v_chunk(tch)
            attn_unit(0, w, u); u += 1
            qk_chunk(1, w)          # Q pair1 block w
            qk_chunk(3, w)          # K pair1 block w
            attn_unit(1, w, u); u += 1
            for c in range(4 * w, 4 * w + 4):
                oproj_chunk(c, wide=(w == NW - 1))

    nc.compile()
    return nc


_NC_CACHE = {}


def _get_nc():
    if "nc" not in _NC_CACHE:
        _NC_CACHE["nc"] = build_nc()
    return _NC_CACHE["nc"]


def make_in_map(core, x, Wq, bq, Wk, bk, Wv, bv, Wp, bp, rope_cache):
    b = core // 4
    hbase = (core % 4) * 4

    pj_np = ml_dtypes.bfloat16
    xTa = np.empty((1024, T), np.float32)
    xTa[:] = np.asarray(x[b], np.float32).T
    x3a = np.ascontiguousarray(
        xTa.reshape(8, 128, T).transpose(1, 0, 2)).astype(pj_np)

    # packed channel order for Q/K: per pair p, heads (hbase+2p, hbase+2p+1),
    # rows [hA_even(32) | hA_odd(32) | hB_even(32) | hB_odd(32)]
    perm = []
    for p in range(2):
        for hh in range(2):
            h = hbase + 2 * p + hh
            perm += [h * HD + 2 * m for m in range(32)]
            perm += [h * HD + 2 * m + 1 for m in range(32)]
    perm = np.asarray(perm)

    wqTa = np.ascontiguousarray(np.asarray(Wq, np.float32)[perm, :].T)
    wq3a = np.ascontiguousarray(
        wqTa.reshape(8, 128, 256).transpose(1, 0, 2)).astype(pj_np)
    wkTa = np.ascontiguousarray(np.asarray(Wk, np.float32)[perm, :].T)
    wk3a = np.ascontiguousarray(
        wkTa.reshape(8, 128, 256).transpose(1, 0, 2)).astype(pj_np)

    chs = np.arange(hbase * HD, hbase * HD + 256)
    wvTa = np.zeros((9 * 128, 260), np.float32)
    WvT = np.asarray(Wv, np.float32)[chs, :].T     # [1024, 256]
    bvc = np.asarray(bv, np.float32)[chs]
    for lh in range(4):
        wvTa[:C, 65 * lh:65 * lh + 64] = WvT[:, 64 * lh:64 * lh + 64]
        wvTa[C, 65 * lh:65 * lh + 64] = bvc[64 * lh:64 * lh + 64]
        wvTa[C, 65 * lh + 64] = 1.0                # ones column (z row)
    wv3a = np.ascontiguousarray(
        wvTa.reshape(9, 128, 260).transpose(1, 0, 2)).astype(pj_np)

    wpTa = np.ascontiguousarray(np.asarray(Wp, np.float32)[:, chs].T)
    wp3a = np.ascontiguousarray(
        wpTa.reshape(2, 128, C).transpose(1, 0, 2)).astype(pj_np)

    bqp = np.asarray(bq, np.float32)[perm].reshape(2, 128).T
    bkp = np.asarray(bk, np.float32)[perm].reshape(2, 128).T
    bqk_a = np.concatenate([bqp, bkp], axis=1)  # [128, 4]

    rc = np.asarray(rope_cache, np.float32)  # [T, 32, 2]
    r = np.arange(128)
    m = r % 32
    sign = np.where((r % 64) < 32, 1.0, -1.0).astype(np.float32)
    cc_a = np.ascontiguousarray(rc[:, m, 0].T)            # [128, T]
    ss_a = np.ascontiguousarray(rc[:, m, 1].T * sign[:, None])

    sl, tl = np.arange(128)[:, None], np.arange(128)[None, :]
    tri_a = np.where(tl >= sl, 0.0, NEG).astype(np.float32)

    return dict(x3=x3a, wq3=wq3a, wk3=wk3a, wv3=wv3a, wp3=wp3a,
                bqk=bqk_a, cc=cc_a, ss=ss_a, tri=tri_a)


def kernel(x, Wq, bq, Wk, bk, Wv, bv, Wp, bp, rope_cache):
    global LAST_EXEC_NS, LAST_RESULTS
    args = (x, Wq, bq, Wk, bk, Wv, bv, Wp, bp, rope_cache)
    nc = _get_nc()
    in_maps = [make_in_map(c, *args) for c in range(NCORES)]
    r = None
    for attempt in range(4):
        try:
            r = run_bass_kernel_spmd(nc, in_maps, list(range(NCORES)))
            break
        except Exception:
            # transient NRT exec-unit errors recover on re-dispatch
            if attempt == 3:
                raise
            time.sleep(5.0 * (attempt + 1))
    LAST_EXEC_NS = r.exec_time_ns
    LAST_RESULTS = r
    out = np.zeros((2, T, C), np.float32)
    for core in range(NCORES):
        out[core // 4] += r.results[core]["out"].astype(np.float32)
    out += np.asarray(bp, np.float32)[None, None, :]
    return out


# revision 6
# speedup vs baseline: 1.3075x; 1.3075x over previous
"""Trainium2 Bass kernel for causal self-attention with RoPE.

Problem: B=2, T=2048, C=1024, H=16 heads, hd=64, fp32, causal, rotary embeddings.

Sharding: 8 cores = 2 batches x 4 head-groups. Core c handles batch c//4 and
heads [4*(c%4), 4*(c%4)+4). Each core computes its heads' Q/K/V projections,
RoPE, causal attention, and a partial output projection over its 256 input
channels; the host sums the 4 partial (fp16) projections per batch and adds
the output bias.

v3 design notes:
  - Attention tiled over FOUR 512-col t-windows, interleaved with projection
    work so ACT (exp, ~86us/core floor) starts early and the PE stays dense
    (HAM keeps the 2.4 GHz clock).
  - Both heads of a pair share one [128,1024] scores PSUM tile -> ONE wide
    ACTIVATE per s-tile covers both heads.
  - att@V + denominator matmuls pack the two heads into PE column groups;
    accumulation uses memset-zeroed PSUM with start=False (no accumulation
    groups), so the interleaved chains in one bank are legal.
  - In-loop emission lags att@V one s-tile behind exp so the PE FIFO never
    blocks on the ACT engine.
  - Inputs arrive in 14 large DMAs (descriptor issue on the sync queue costs
    ~0.6us each); rope's 32-row swap uses DVE copies, not DMA.
  - fp16 output partials, one wide DMA per 128-row t-chunk.
"""

import os
import time
from contextlib import ExitStack

import ml_dtypes
import numpy as np

import concourse.bass as bass
import concourse.tile as tile
from concourse import bacc, library_config, mybir
from concourse.bass_utils import run_bass_kernel_spmd

F32 = mybir.dt.float32
F16 = mybir.dt.float16
BF16 = mybir.dt.bfloat16

T = 2048
C = 1024
HD = 64
NCORES = 8
NEG = -1e10
NW = 4            # t-windows of 512
WW = 512          # window width

AF = mybir.ActivationFunctionType
ALU = mybir.AluOpType

LAST_EXEC_NS = None
LAST_RESULTS = None


def build_nc():
    nc = bacc.Bacc("TRN2", target_bir_lowering=False, debug=False)

    x3 = nc.dram_tensor("x3", [128, 8, T], BF16, kind="ExternalInput").ap()
    wq3 = nc.dram_tensor("wq3", [128, 8, 256], BF16, kind="ExternalInput").ap()
    wk3 = nc.dram_tensor("wk3", [128, 8, 256], BF16, kind="ExternalInput").ap()
    wv3 = nc.dram_tensor("wv3", [128, 9, 260], BF16, kind="ExternalInput").ap()
    wp3 = nc.dram_tensor("wp3", [128, 2, C], BF16, kind="ExternalInput").ap()
    bqk = nc.dram_tensor("bqk", [128, 4], F32, kind="ExternalInput").ap()
    cc_d = nc.dram_tensor("cc", [128, T], F32, kind="ExternalInput").ap()
    ss_d = nc.dram_tensor("ss", [128, T], F32, kind="ExternalInput").ap()
    tri_d = nc.dram_tensor("tri", [128, 128], F32, kind="ExternalInput").ap()
    out_d = nc.dram_tensor("out", [T, C], F16, kind="ExternalOutput").ap()

    with tile.TileContext(nc) as tc, ExitStack() as ctx:
        consts = ctx.enter_context(tc.tile_pool(name="consts", bufs=1))

        # ---- persistent SBUF tiles ----
        cc_sb = consts.tile([128, T], F32)
        ss_sb = consts.tile([128, T], F32)
        tri_sb = consts.tile([128, 128], F32)
        bqk_sb = consts.tile([128, 4], F32)
        x1 = consts.tile([1, T], BF16)          # ones row (V bias matmul)
        wsrc = consts.tile([128, 512], BF16)    # warmup matmul source

        xa = consts.tile([128, 8 * T], BF16, name="xa")
        xv = xa.rearrange("p (j t) -> p j t", t=T)
        wqa = consts.tile([128, 8 * 256], BF16, name="wqa")
        wqv = wqa.rearrange("p (j c) -> p j c", c=256)
        wka = consts.tile([128, 8 * 256], BF16, name="wka")
        wkv = wka.rearrange("p (j c) -> p j c", c=256)
        wva = consts.tile([128, 9 * 260], BF16, name="wva")
        wvv = wva.rearrange("p (j c) -> p j c", c=260)
        wpa = consts.tile([128, 2 * C], BF16, name="wpa")
        wpv = wpa.rearrange("p (q c) -> p q c", c=C)

        # rotated Q^T / K^T, one [128, T] tile per head pair:
        # 0,1 = Q pair0/1; 2,3 = K pair0/1. rows [hA(64) | hB(64)].
        qkt = [consts.tile([128, T], BF16, name=f"qkt{i}") for i in range(4)]
        # V' per s-tile: [128 s, 256] = 4 heads x 64 dims
        vp = [consts.tile([128, 260], BF16, name=f"vp{i}") for i in range(16)]
        # scaled attention outputs (d-major), one [128, T] per pair
        usc = [consts.tile([128, T], BF16, name=f"usc{p}") for p in range(2)]

        # ---- PSUM pools (8 banks total) ----
        sp = ctx.enter_context(
            tc.tile_pool(name="sp", bufs=2, space="PSUM"))      # 4 banks
        yp = ctx.enter_context(
            tc.tile_pool(name="yp", bufs=3, space="PSUM"))      # 3 banks
        qp = ctx.enter_context(
            tc.tile_pool(name="qp", bufs=1, space="PSUM"))      # 1 bank

        # ---- SBUF working pools ----
        etp = ctx.enter_context(tc.tile_pool(name="etp", bufs=6))
        rp = ctx.enter_context(tc.tile_pool(name="rp", bufs=2))
        dvp = ctx.enter_context(tc.tile_pool(name="dvp", bufs=2))
        ost = ctx.enter_context(tc.tile_pool(name="ost", bufs=4))

        # ================= t=0: warmup + setup =================
        nc.vector.memset(wsrc[:], 0.0)
        warm = sp.tile([128, 1024], F32, tag="s", name="warm")
        for r in range(14):
            nc.tensor.matmul(warm[:, 0:512], wsrc[:, 0:128], wsrc[:],
                             start=True, stop=True)
        nc.vector.memset(x1[:], 1.0)
        # preload the exp ACT table set early (table load ~2.7us)
        dummy_et = consts.tile([1, 128], BF16, name="dummy_et")
        nc.scalar.activation(dummy_et[:], wsrc[0:1, 0:128], AF.Exp, scale=0.125)
        nc.gpsimd.load_library(library_config.attn)

        # ================= DMA schedule (priority = order) =================
        def wsl(w):
            return slice(WW * w, WW * (w + 1))

        nc.sync.dma_start(wqa[:], wq3[:, :, :])
        nc.sync.dma_start(xv[:, :, wsl(0)], x3[:, :, wsl(0)])
        nc.sync.dma_start(bqk_sb[:], bqk[:])
        nc.sync.dma_start(cc_sb[:, 0:1024], cc_d[:, 0:1024])
        nc.sync.dma_start(ss_sb[:, 0:1024], ss_d[:, 0:1024])
        nc.sync.dma_start(wka[:], wk3[:, :, :])
        nc.sync.dma_start(tri_sb[:], tri_d[:])
        nc.sync.dma_start(wva[:], wv3[:, :, :])
        nc.sync.dma_start(xv[:, :, wsl(1)], x3[:, :, wsl(1)])
        nc.sync.dma_start(xv[:, :, wsl(2)], x3[:, :, wsl(2)])
        nc.sync.dma_start(cc_sb[:, 1024:2048], cc_d[:, 1024:2048])
        nc.sync.dma_start(ss_sb[:, 1024:2048], ss_d[:, 1024:2048])
        nc.sync.dma_start(xv[:, :, wsl(3)], x3[:, :, wsl(3)])
        nc.sync.dma_start(wpa[:], wp3[:, :, :])

        # ================= building blocks =================
        def qk_chunk(t_idx, w):
            """Project + rope one 512-col block of qkt[t_idx]."""
            wv_ = wqv if t_idx < 2 else wkv
            pr = t_idx % 2
            bcol = (2 if t_idx >= 2 else 0) + pr
            bias = bqk_sb[:, bcol:bcol + 1]
            ps = qp.tile([128, WW], F32, tag="qp", name=f"qk{t_idx}_{w}")
            for j in range(8):
                nc.tensor.matmul(
                    ps[:], wv_[:, j, 128 * pr:128 * (pr + 1)],
                    xv[:, j, wsl(w)], start=(j == 0), stop=(j == 7))
            qs = rp.tile([128, WW], F32, tag="qs")
            nc.vector.tensor_copy(qs[:], ps[:])   # frees the qp bank fast
            p1 = rp.tile([128, WW], F32, tag="p1")
            p2 = rp.tile([128, WW], F32, tag="p2")
            p2s = rp.tile([128, WW], F32, tag="p2s")
            nc.vector.scalar_tensor_tensor(
                out=p1[:], in0=qs[:], scalar=bias,
                in1=cc_sb[:, wsl(w)], op0=ALU.add, op1=ALU.mult)
            nc.vector.scalar_tensor_tensor(
                out=p2[:], in0=qs[:], scalar=bias,
                in1=ss_sb[:, wsl(w)], op0=ALU.add, op1=ALU.mult)
            for r in range(4):
                src = slice(32 * (r ^ 1), 32 * (r ^ 1) + 32)
                dst = slice(32 * r, 32 * r + 32)
                nc.sync.dma_start(p2s[dst, :], p2[src, :])
            nc.vector.tensor_add(qkt[t_idx][:, wsl(w)], p1[:], p2s[:])

        def v_chunk(tch):
            """V' for s-tile tch: [128 s, 256] = 4 heads x 64 dims."""
            vr = qp.tile([128, 260], F32, tag="qp", name=f"vr{tch}")
            tsl = slice(128 * tch, 128 * (tch + 1))
            for j in range(8):
                nc.tensor.matmul(vr[:], xv[:, j, tsl], wvv[:, j, :],
                                 start=(j == 0), stop=False)
            nc.tensor.matmul(vr[:], x1[:, tsl], wvv[0:1, 8, :],
                             start=False, stop=True)
            nc.vector.tensor_copy(vp[tch][:], vr[:])

        def attn_unit(p, w, u):
            """Attention for head-pair p over t-window w (s-tiles 0..4w+3)."""
            qt, kt = qkt[p], qkt[2 + p]
            ni = 4 * w + 4
            # per-head [65, WW] accumulators: rows 0:64 = y, row 64 = z
            # (ones column in V'), one PSUM bank each.
            yz = [yp.tile([65, WW], F32, tag="y", name=f"yz{p}_{w}_{hs}")
                  for hs in range(2)]
            ets = {}

            def scores_exp(i):
                lo = max(0, 128 * i - WW * w)
                sc = sp.tile([128, 1024], F32, tag="s", name=f"s{p}_{w}_{i}")
                # scores: A (rows 0:64) into cols lo:512, B (64:128) into
                # cols 512:1024 (untrimmed so the exp region is contiguous)
                nc.tensor.matmul(
                    sc[:, lo:WW],
                    kt[0:64, 128 * i:128 * (i + 1)],
                    qt[0:64, WW * w + lo:WW * (w + 1)],
                    start=True, stop=True)
                nc.tensor.matmul(
                    sc[:, WW:2 * WW],
                    kt[64:128, 128 * i:128 * (i + 1)],
                    qt[64:128, wsl(w)],
                    start=True, stop=True)
                if i >= 4 * w:  # diagonal tile: causal mask both heads
                    nc.vector.tensor_add(sc[:, lo:lo + 128],
                                         sc[:, lo:lo + 128], tri_sb[:])
                    nc.vector.tensor_add(sc[:, WW + lo:WW + lo + 128],
                                         sc[:, WW + lo:WW + lo + 128],
                                         tri_sb[:])
                et = etp.tile([128, 1024], BF16, tag="e", name=f"e{p}_{w}_{i}")
                nc.scalar.activation(et[:, lo:], sc[:, lo:], AF.Exp,
                                     scale=0.125)
                ets[i] = (et, lo)

            def yz_mm(i):
                et, lo = ets.pop(i)
                st, sp_ = (i == 0), (i == ni - 1)
                nc.tensor.matmul(yz[0][:, lo:],
                                 vp[i][:, 130 * p:130 * p + 65],
                                 et[:, lo:WW], start=st, stop=sp_)
                nc.tensor.matmul(yz[1][:, lo:],
                                 vp[i][:, 130 * p + 65:130 * p + 130],
                                 et[:, WW + lo:2 * WW], start=st, stop=sp_)

            # software pipeline: att@V lags exp by one s-tile so the PE FIFO
            # always has ready work while ACT computes the current exp.
            for i in range(ni):
                scores_exp(i)
                if i > 0:
                    yz_mm(i - 1)
            yz_mm(ni - 1)

            # normalize: usc[p][64hs:, w] = y * (1/z) broadcast per head
            for hs in range(2):
                zrow = dvp.tile([1, WW], F32, tag="zrow", name=f"zr{p}_{w}{hs}")
                rzr = dvp.tile([1, WW], F32, tag="rzr", name=f"rz{p}_{w}{hs}")
                rzb = dvp.tile([64, WW], F32, tag="rzb", name=f"rzb{p}_{w}{hs}")
                nc.vector.tensor_copy(zrow[:], yz[hs][64:65, :])
                nc.vector.reciprocal_approx_fast(rzr[:], zrow[:])
                nc.gpsimd.partition_broadcast(rzb[:], rzr[:])
                nc.vector.tensor_mul(usc[p][64 * hs:64 * (hs + 1), wsl(w)],
                                     yz[hs][0:64, :], rzb[:])

        def oproj_chunk(c, wide):
            """Output projection for t-chunk c (128 t-cols)."""
            tsl = slice(128 * c, 128 * (c + 1))
            st = ost.tile([128, C], F16, tag="o", name=f"os{c}")
            if not wide:
                for cg in range(2):
                    csl = slice(512 * cg, 512 * (cg + 1))
                    ps = qp.tile([128, 512], F32, tag="qp", name=f"op{c}_{cg}")
                    for pq in range(2):
                        nc.tensor.matmul(ps[:], usc[pq][:, tsl],
                                         wpv[:, pq, csl],
                                         start=(pq == 0), stop=(pq == 1))
                    nc.vector.tensor_copy(st[:, csl], ps[:])
            else:
                ps = sp.tile([128, 1024], F32, tag="s", name=f"opw{c}")
                for cg in range(2):
                    csl = slice(512 * cg, 512 * (cg + 1))
                    for pq in range(2):
                        nc.tensor.matmul(ps[:, csl], usc[pq][:, tsl],
                                         wpv[:, pq, csl],
                                         start=(pq == 0), stop=(pq == 1))
                nc.vector.tensor_copy(st[:], ps[:])
            nc.sync.dma_start(out_d[tsl, :], st[:])

        # ================= emission schedule =================
        # window-0 projections up front; thereafter window w+1's
        # projections are emitted inside window w's attention section so the
        # rope chain completes while the PE/ACT crunch the previous window.
        u = 0
        qk_chunk(0, 0); qk_chunk(2, 0)
        for tch in range(0, 4):
            v_chunk(tch)
        qk_chunk(1, 0); qk_chunk(3, 0)
        for w in range(NW):
            attn_unit(0, w, u); u += 1
            if w + 1 < NW:
                qk_chunk(0, w + 1); qk_chunk(2, w + 1)
            attn_unit(1, w, u); u += 1
            if w + 1 < NW:
                qk_chunk(1, w + 1); qk_chunk(3, w + 1)
                for tch in range(4 * w + 4, 4 * w + 8):
                    v_chunk(tch)
            for c in range(4 * w, 4 * w + 4):
                oproj_chunk(c, wide=(w == NW - 1))

    nc.compile()
    return nc


_NC_CACHE = {}


def _get_nc():
    if "nc" not in _NC_CACHE:
        _NC_CACHE["nc"] = build_nc()
    return _NC_CACHE["nc"]


def make_in_map(core, x, Wq, bq, Wk, bk, Wv, bv, Wp, bp, rope_cache):
    b = core // 4
    hbase = (core % 4) * 4

    pj_np = ml_dtypes.bfloat16
    xTa = np.empty((1024, T), np.float32)
    xTa[:] = np.asarray(x[b], np.float32).T
    x3a = np.ascontiguousarray(
        xTa.reshape(8, 128, T).transpose(1, 0, 2)).astype(pj_np)

    # packed channel order for Q/K: per pair p, heads (hbase+2p, hbase+2p+1),
    # rows [hA_even(32) | hA_odd(32) | hB_even(32) | hB_odd(32)]
    perm = []
    for p in range(2):
        for hh in range(2):
            h = hbase + 2 * p + hh
            perm += [h * HD + 2 * m for m in range(32)]
            perm += [h * HD + 2 * m + 1 for m in range(32)]
    perm = np.asarray(perm)

    wqTa = np.ascontiguousarray(np.asarray(Wq, np.float32)[perm, :].T)
    wq3a = np.ascontiguousarray(
        wqTa.reshape(8, 128, 256).transpose(1, 0, 2)).astype(pj_np)
    wkTa = np.ascontiguousarray(np.asarray(Wk, np.float32)[perm, :].T)
    wk3a = np.ascontiguousarray(
        wkTa.reshape(8, 128, 256).transpose(1, 0, 2)).astype(pj_np)

    chs = np.arange(hbase * HD, hbase * HD + 256)
    wvTa = np.zeros((9 * 128, 260), np.float32)
    WvT = np.asarray(Wv, np.float32)[chs, :].T     # [1024, 256]
    bvc = np.asarray(bv, np.float32)[chs]
    for lh in range(4):
        wvTa[:C, 65 * lh:65 * lh + 64] = WvT[:, 64 * lh:64 * lh + 64]
        wvTa[C, 65 * lh:65 * lh + 64] = bvc[64 * lh:64 * lh + 64]
        wvTa[C, 65 * lh + 64] = 1.0                # ones column (z row)
    wv3a = np.ascontiguousarray(
        wvTa.reshape(9, 128, 260).transpose(1, 0, 2)).astype(pj_np)

    wpTa = np.ascontiguousarray(np.asarray(Wp, np.float32)[:, chs].T)
    wp3a = np.ascontiguousarray(
        wpTa.reshape(2, 128, C).transpose(1, 0, 2)).astype(pj_np)

    bqp = np.asarray(bq, np.float32)[perm].reshape(2, 128).T
    bkp = np.asarray(bk, np.float32)[perm].reshape(2, 128).T
    bqk_a = np.concatenate([bqp, bkp], axis=1)  # [128, 4]

    rc = np.asarray(rope_cache, np.float32)  # [T, 32, 2]
    r = np.arange(128)
    m = r % 32
    sign = np.where((r % 64) < 32, 1.0, -1.0).astype(np.float32)
    cc_a = np.ascontiguousarray(rc[:, m, 0].T)            # [128, T]
    ss_a = np.ascontiguousarray(rc[:, m, 1].T * sign[:, None])

    sl, tl = np.arange(128)[:, None], np.arange(128)[None, :]
    tri_a = np.where(tl >= sl, 0.0, NEG).astype(np.float32)

    return dict(x3=x3a, wq3=wq3a, wk3=wk3a, wv3=wv3a, wp3=wp3a,
                bqk=bqk_a, cc=cc_a, ss=ss_a, tri=tri_a)


def kernel(x, Wq, bq, Wk, bk, Wv, bv, Wp, bp, rope_cache):
    global LAST_EXEC_NS, LAST_RESULTS
    args = (x, Wq, bq, Wk, bk, Wv, bv, Wp, bp, rope_cache)
    nc = _get_nc()
    in_maps = [make_in_map(c, *args) for c in range(NCORES)]
    r = None
    for attempt in range(4):
        try:
            r = run_bass_kernel_spmd(nc, in_maps, list(range(NCORES)))
            break
        except Exception:
            # transient NRT exec-unit errors recover on re-dispatch
            if attempt == 3:
                raise
            time.sleep(5.0 * (attempt + 1))
    LAST_EXEC_NS = r.exec_time_ns
    LAST_RESULTS = r
    out = np.zeros((2, T, C), np.float32)
    for core in range(NCORES):
        out[core // 4] += r.results[core]["out"].astype(np.float32)
    out += np.asarray(bp, np.float32)[None, None, :]
    return out


# revision 7
# speedup vs baseline: 1.4112x; 1.0793x over previous
"""Trainium2 Bass kernel for causal self-attention with RoPE.

Problem: B=2, T=2048, C=1024, H=16 heads, hd=64, fp32, causal, rotary embeddings.

Sharding: 8 cores = 2 batches x 4 head-groups. Core c handles batch c//4 and
heads [4*(c%4), 4*(c%4)+4). Each core computes its heads' Q/K/V projections,
RoPE, causal attention, and a partial output projection over its 256 input
channels; the host sums the 4 partial (fp16) projections per batch and adds
the output bias.

v3 design notes:
  - Attention tiled over FOUR 512-col t-windows, interleaved with projection
    work so ACT (exp, ~86us/core floor) starts early and the PE stays dense
    (HAM keeps the 2.4 GHz clock).
  - Both heads of a pair share one [128,1024] scores PSUM tile -> ONE wide
    ACTIVATE per s-tile covers both heads.
  - att@V + denominator matmuls pack the two heads into PE column groups;
    accumulation uses memset-zeroed PSUM with start=False (no accumulation
    groups), so the interleaved chains in one bank are legal.
  - In-loop emission lags att@V one s-tile behind exp so the PE FIFO never
    blocks on the ACT engine.
  - Inputs arrive in 14 large DMAs (descriptor issue on the sync queue costs
    ~0.6us each); rope's 32-row swap uses DVE copies, not DMA.
  - fp16 output partials, one wide DMA per 128-row t-chunk.
"""

import os
import time
from contextlib import ExitStack

import ml_dtypes
import numpy as np

import concourse.bass as bass
import concourse.tile as tile
from concourse import bacc, library_config, mybir
from concourse.bass_utils import run_bass_kernel_spmd

F32 = mybir.dt.float32
F16 = mybir.dt.float16
BF16 = mybir.dt.bfloat16

T = 2048
C = 1024
HD = 64
NCORES = 8
NEG = -1e10
NW = 4            # t-windows of 512
WW = 512          # window width

AF = mybir.ActivationFunctionType
ALU = mybir.AluOpType

LAST_EXEC_NS = None
LAST_RESULTS = None


def build_nc():
    nc = bacc.Bacc("TRN2", target_bir_lowering=False, debug=False)

    x3 = nc.dram_tensor("x3", [128, 8, T], BF16, kind="ExternalInput").ap()
    wq3 = nc.dram_tensor("wq3", [128, 8, 256], BF16, kind="ExternalInput").ap()
    wk3 = nc.dram_tensor("wk3", [128, 8, 256], BF16, kind="ExternalInput").ap()
    wv3 = nc.dram_tensor("wv3", [128, 9, 260], BF16, kind="ExternalInput").ap()
    wp3 = nc.dram_tensor("wp3", [128, 2, C], BF16, kind="ExternalInput").ap()
    bqk = nc.dram_tensor("bqk", [128, 4], F32, kind="ExternalInput").ap()
    cc_d = nc.dram_tensor("cc", [128, T], F32, kind="ExternalInput").ap()
    ss_d = nc.dram_tensor("ss", [128, T], F32, kind="ExternalInput").ap()
    tri_d = nc.dram_tensor("tri", [128, 128], F32, kind="ExternalInput").ap()
    out_d = nc.dram_tensor("out", [T, C], F16, kind="ExternalOutput").ap()

    with tile.TileContext(nc) as tc, ExitStack() as ctx:
        consts = ctx.enter_context(tc.tile_pool(name="consts", bufs=1))

        # ---- persistent SBUF tiles ----
        cc_sb = consts.tile([128, T], F32)
        ss_sb = consts.tile([128, T], F32)
        tri_sb = consts.tile([128, 128], F32)
        bqk_sb = consts.tile([128, 4], F32)
        x1 = consts.tile([1, T], BF16)          # ones row (V bias matmul)
        wsrc = consts.tile([128, 512], BF16)    # warmup matmul source

        xa = consts.tile([128, 8 * T], BF16, name="xa")
        xv = xa.rearrange("p (j t) -> p j t", t=T)
        wqa = consts.tile([128, 8 * 256], BF16, name="wqa")
        wqv = wqa.rearrange("p (j c) -> p j c", c=256)
        wka = consts.tile([128, 8 * 256], BF16, name="wka")
        wkv = wka.rearrange("p (j c) -> p j c", c=256)
        wva = consts.tile([128, 9 * 260], BF16, name="wva")
        wvv = wva.rearrange("p (j c) -> p j c", c=260)
        wpa = consts.tile([128, 2 * C], BF16, name="wpa")
        wpv = wpa.rearrange("p (q c) -> p q c", c=C)

        # rotated Q^T / K^T, one [128, T] tile per head pair:
        # 0,1 = Q pair0/1; 2,3 = K pair0/1. rows [hA(64) | hB(64)].
        qkt = [consts.tile([128, T], BF16, name=f"qkt{i}") for i in range(4)]
        # V' per s-tile: [128 s, 256] = 4 heads x 64 dims
        vp = [consts.tile([128, 260], BF16, name=f"vp{i}") for i in range(16)]
        # scaled attention outputs (d-major), one [128, T] per pair
        usc = [consts.tile([128, T], BF16, name=f"usc{p}") for p in range(2)]

        # ---- PSUM pools (8 banks total) ----
        sp = ctx.enter_context(
            tc.tile_pool(name="sp", bufs=2, space="PSUM"))      # 4 banks
        yp = ctx.enter_context(
            tc.tile_pool(name="yp", bufs=3, space="PSUM"))      # 3 banks
        qp = ctx.enter_context(
            tc.tile_pool(name="qp", bufs=1, space="PSUM"))      # 1 bank

        # ---- SBUF working pools ----
        etp = ctx.enter_context(tc.tile_pool(name="etp", bufs=8))
        rp = ctx.enter_context(tc.tile_pool(name="rp", bufs=2))
        dvp = ctx.enter_context(tc.tile_pool(name="dvp", bufs=2))
        ost = ctx.enter_context(tc.tile_pool(name="ost", bufs=4))

        # ================= t=0: warmup + setup =================
        nc.vector.memset(wsrc[:], 0.0)
        warm = sp.tile([128, 1024], F32, tag="s", name="warm")
        for r in range(14):
            nc.tensor.matmul(warm[:, 0:512], wsrc[:, 0:128], wsrc[:],
                             start=True, stop=True)
        nc.vector.memset(x1[:], 1.0)
        # preload the exp ACT table set early (table load ~2.7us)
        dummy_et = consts.tile([1, 128], BF16, name="dummy_et")
        nc.scalar.activation(dummy_et[:], wsrc[0:1, 0:128], AF.Exp, scale=0.125)
        nc.gpsimd.load_library(library_config.attn)

        # ================= DMA schedule (priority = order) =================
        def wsl(w):
            return slice(WW * w, WW * (w + 1))

        nc.sync.dma_start(wqa[:], wq3[:, :, :])
        nc.sync.dma_start(xv[:, :, wsl(0)], x3[:, :, wsl(0)])
        nc.sync.dma_start(bqk_sb[:], bqk[:])
        nc.sync.dma_start(cc_sb[:, 0:1024], cc_d[:, 0:1024])
        nc.sync.dma_start(ss_sb[:, 0:1024], ss_d[:, 0:1024])
        nc.sync.dma_start(wka[:], wk3[:, :, :])
        nc.sync.dma_start(tri_sb[:], tri_d[:])
        nc.sync.dma_start(wva[:], wv3[:, :, :])
        nc.sync.dma_start(xv[:, :, wsl(1)], x3[:, :, wsl(1)])
        nc.sync.dma_start(xv[:, :, wsl(2)], x3[:, :, wsl(2)])
        nc.sync.dma_start(cc_sb[:, 1024:2048], cc_d[:, 1024:2048])
        nc.sync.dma_start(ss_sb[:, 1024:2048], ss_d[:, 1024:2048])
        nc.sync.dma_start(xv[:, :, wsl(3)], x3[:, :, wsl(3)])
        nc.sync.dma_start(wpa[:], wp3[:, :, :])

        # ================= building blocks =================
        def qk_chunk(t_idx, w):
            """Project + rope one 512-col block of qkt[t_idx]."""
            wv_ = wqv if t_idx < 2 else wkv
            pr = t_idx % 2
            bcol = (2 if t_idx >= 2 else 0) + pr
            bias = bqk_sb[:, bcol:bcol + 1]
            ps = qp.tile([128, WW], F32, tag="qp", name=f"qk{t_idx}_{w}")
            for j in range(8):
                nc.tensor.matmul(
                    ps[:], wv_[:, j, 128 * pr:128 * (pr + 1)],
                    xv[:, j, wsl(w)], start=(j == 0), stop=(j == 7))
            qs = rp.tile([128, WW], F32, tag="qs")
            nc.vector.tensor_copy(qs[:], ps[:])   # frees the qp bank fast
            p1 = rp.tile([128, WW], F32, tag="p1")
            p2 = rp.tile([128, WW], F32, tag="p2")
            p2s = rp.tile([128, WW], F32, tag="p2s")
            nc.vector.scalar_tensor_tensor(
                out=p1[:], in0=qs[:], scalar=bias,
                in1=cc_sb[:, wsl(w)], op0=ALU.add, op1=ALU.mult)
            nc.vector.scalar_tensor_tensor(
                out=p2[:], in0=qs[:], scalar=bias,
                in1=ss_sb[:, wsl(w)], op0=ALU.add, op1=ALU.mult)
            for r in range(4):
                src = slice(32 * (r ^ 1), 32 * (r ^ 1) + 32)
                dst = slice(32 * r, 32 * r + 32)
                nc.sync.dma_start(p2s[dst, :], p2[src, :])
            nc.vector.tensor_add(qkt[t_idx][:, wsl(w)], p1[:], p2s[:])

        def v_chunk(tch):
            """V' for s-tile tch: [128 s, 256] = 4 heads x 64 dims."""
            vr = qp.tile([128, 260], F32, tag="qp", name=f"vr{tch}")
            tsl = slice(128 * tch, 128 * (tch + 1))
            for j in range(8):
                nc.tensor.matmul(vr[:], xv[:, j, tsl], wvv[:, j, :],
                                 start=(j == 0), stop=False)
            nc.tensor.matmul(vr[:], x1[:, tsl], wvv[0:1, 8, :],
                             start=False, stop=True)
            nc.vector.tensor_copy(vp[tch][:], vr[:])

        def attn_unit(p, w, u):
            """Attention for head-pair p over t-window w (s-tiles 0..4w+3)."""
            qt, kt = qkt[p], qkt[2 + p]
            ni = 4 * w + 4
            # per-head [65, WW] accumulators: rows 0:64 = y, row 64 = z
            # (ones column in V'), one PSUM bank each.
            yz = [yp.tile([65, WW], F32, tag="y", name=f"yz{p}_{w}_{hs}")
                  for hs in range(2)]
            ets = {}

            def scores_exp(i):
                lo = max(0, 128 * i - WW * w)
                sc = sp.tile([128, 1024], F32, tag="s", name=f"s{p}_{w}_{i}")
                # scores: A (rows 0:64) into cols lo:512, B (64:128) into
                # cols 512:1024 (untrimmed so the exp region is contiguous)
                nc.tensor.matmul(
                    sc[:, lo:WW],
                    kt[0:64, 128 * i:128 * (i + 1)],
                    qt[0:64, WW * w + lo:WW * (w + 1)],
                    start=True, stop=True)
                nc.tensor.matmul(
                    sc[:, WW:2 * WW],
                    kt[64:128, 128 * i:128 * (i + 1)],
                    qt[64:128, wsl(w)],
                    start=True, stop=True)
                if i >= 4 * w:  # diagonal tile: causal mask both heads
                    nc.vector.tensor_add(sc[:, lo:lo + 128],
                                         sc[:, lo:lo + 128], tri_sb[:])
                    nc.vector.tensor_add(sc[:, WW + lo:WW + lo + 128],
                                         sc[:, WW + lo:WW + lo + 128],
                                         tri_sb[:])
                et = etp.tile([128, 1024], BF16, tag="e", name=f"e{p}_{w}_{i}")
                nc.scalar.activation(et[:, lo:], sc[:, lo:], AF.Exp,
                                     scale=0.125)
                ets[i] = (et, lo)

            def yz_mm(i):
                et, lo = ets.pop(i)
                st, sp_ = (i == 0), (i == ni - 1)
                nc.tensor.matmul(yz[0][:, lo:],
                                 vp[i][:, 130 * p:130 * p + 65],
                                 et[:, lo:WW], start=st, stop=sp_)
                nc.tensor.matmul(yz[1][:, lo:],
                                 vp[i][:, 130 * p + 65:130 * p + 130],
                                 et[:, WW + lo:2 * WW], start=st, stop=sp_)

            # software pipeline: att@V lags exp by one s-tile so the PE FIFO
            # always has ready work while ACT computes the current exp.
            for i in range(ni):
                scores_exp(i)
                if i > 1:
                    yz_mm(i - 2)
            yz_mm(ni - 2)
            yz_mm(ni - 1)

            # normalize: usc[p][64hs:, w] = y * (1/z) broadcast per head
            for hs in range(2):
                zrow = dvp.tile([1, WW], F32, tag="zrow", name=f"zr{p}_{w}{hs}")
                rzr = dvp.tile([1, WW], F32, tag="rzr", name=f"rz{p}_{w}{hs}")
                rzb = dvp.tile([64, WW], F32, tag="rzb", name=f"rzb{p}_{w}{hs}")
                nc.vector.tensor_copy(zrow[:], yz[hs][64:65, :])
                nc.vector.reciprocal_approx_fast(rzr[:], zrow[:])
                nc.gpsimd.partition_broadcast(rzb[:], rzr[:])
                nc.vector.tensor_mul(usc[p][64 * hs:64 * (hs + 1), wsl(w)],
                                     yz[hs][0:64, :], rzb[:])

        def oproj_chunk(c, wide):
            """Output projection for t-chunk c (128 t-cols)."""
            tsl = slice(128 * c, 128 * (c + 1))
            st = ost.tile([128, C], F16, tag="o", name=f"os{c}")
            if not wide:
                for cg in range(2):
                    csl = slice(512 * cg, 512 * (cg + 1))
                    ps = qp.tile([128, 512], F32, tag="qp", name=f"op{c}_{cg}")
                    for pq in range(2):
                        nc.tensor.matmul(ps[:], usc[pq][:, tsl],
                                         wpv[:, pq, csl],
                                         start=(pq == 0), stop=(pq == 1))
                    nc.vector.tensor_copy(st[:, csl], ps[:])
            else:
                ps = sp.tile([128, 1024], F32, tag="s", name=f"opw{c}")
                for cg in range(2):
                    csl = slice(512 * cg, 512 * (cg + 1))
                    for pq in range(2):
                        nc.tensor.matmul(ps[:, csl], usc[pq][:, tsl],
                                         wpv[:, pq, csl],
                                         start=(pq == 0), stop=(pq == 1))
                nc.vector.tensor_copy(st[:], ps[:])
            nc.sync.dma_start(out_d[tsl, :], st[:])

        # ================= emission schedule =================
        # window-0 projections up front; thereafter window w+1's
        # projections are emitted inside window w's attention section so the
        # rope chain completes while the PE/ACT crunch the previous window.
        u = 0
        qk_chunk(0, 0); qk_chunk(2, 0)
        for tch in range(0, 4):
            v_chunk(tch)
        qk_chunk(1, 0); qk_chunk(3, 0)
        for w in range(NW):
            attn_unit(0, w, u); u += 1
            if w + 1 < NW:
                qk_chunk(0, w + 1); qk_chunk(2, w + 1)
            attn_unit(1, w, u); u += 1
            if w + 1 < NW:
                qk_chunk(1, w + 1); qk_chunk(3, w + 1)
                for tch in range(4 * w + 4, 4 * w + 8):
                    v_chunk(tch)
            for c in range(4 * w, 4 * w + 4):
                oproj_chunk(c, wide=(w == NW - 1))

    nc.compile()
    return nc


_NC_CACHE = {}


def _get_nc():
    if "nc" not in _NC_CACHE:
        _NC_CACHE["nc"] = build_nc()
    return _NC_CACHE["nc"]


def make_in_map(core, x, Wq, bq, Wk, bk, Wv, bv, Wp, bp, rope_cache):
    b = core // 4
    hbase = (core % 4) * 4

    pj_np = ml_dtypes.bfloat16
    xTa = np.empty((1024, T), np.float32)
    xTa[:] = np.asarray(x[b], np.float32).T
    x3a = np.ascontiguousarray(
        xTa.reshape(8, 128, T).transpose(1, 0, 2)).astype(pj_np)

    # packed channel order for Q/K: per pair p, heads (hbase+2p, hbase+2p+1),
    # rows [hA_even(32) | hA_odd(32) | hB_even(32) | hB_odd(32)]
    perm = []
    for p in range(2):
        for hh in range(2):
            h = hbase + 2 * p + hh
            perm += [h * HD + 2 * m for m in range(32)]
            perm += [h * HD + 2 * m + 1 for m in range(32)]
    perm = np.asarray(perm)

    wqTa = np.ascontiguousarray(np.asarray(Wq, np.float32)[perm, :].T)
    wq3a = np.ascontiguousarray(
        wqTa.reshape(8, 128, 256).transpose(1, 0, 2)).astype(pj_np)
    wkTa = np.ascontiguousarray(np.asarray(Wk, np.float32)[perm, :].T)
    wk3a = np.ascontiguousarray(
        wkTa.reshape(8, 128, 256).transpose(1, 0, 2)).astype(pj_np)

    chs = np.arange(hbase * HD, hbase * HD + 256)
    wvTa = np.zeros((9 * 128, 260), np.float32)
    WvT = np.asarray(Wv, np.float32)[chs, :].T     # [1024, 256]
    bvc = np.asarray(bv, np.float32)[chs]
    for lh in range(4):
        wvTa[:C, 65 * lh:65 * lh + 64] = WvT[:, 64 * lh:64 * lh + 64]
        wvTa[C, 65 * lh:65 * lh + 64] = bvc[64 * lh:64 * lh + 64]
        wvTa[C, 65 * lh + 64] = 1.0                # ones column (z row)
    wv3a = np.ascontiguousarray(
        wvTa.reshape(9, 128, 260).transpose(1, 0, 2)).astype(pj_np)

    wpTa = np.ascontiguousarray(np.asarray(Wp, np.float32)[:, chs].T)
    wp3a = np.ascontiguousarray(
        wpTa.reshape(2, 128, C).transpose(1, 0, 2)).astype(pj_np)

    bqp = np.asarray(bq, np.float32)[perm].reshape(2, 128).T
    bkp = np.asarray(bk, np.float32)[perm].reshape(2, 128).T
    bqk_a = np.concatenate([bqp, bkp], axis=1)  # [128, 4]

    rc = np.asarray(rope_cache, np.float32)  # [T, 32, 2]
    r = np.arange(128)
    m = r % 32
    sign = np.where((r % 64) < 32, 1.0, -1.0).astype(np.float32)
    cc_a = np.ascontiguousarray(rc[:, m, 0].T)            # [128, T]
    ss_a = np.ascontiguousarray(rc[:, m, 1].T * sign[:, None])

    sl, tl = np.arange(128)[:, None], np.arange(128)[None, :]
    tri_a = np.where(tl >= sl, 0.0, NEG).astype(np.float32)

    return dict(x3=x3a, wq3=wq3a, wk3=wk3a, wv3=wv3a, wp3=wp3a,
                bqk=bqk_a, cc=cc_a, ss=ss_a, tri=tri_a)


def kernel(x, Wq, bq, Wk, bk, Wv, bv, Wp, bp, rope_cache):
    global LAST_EXEC_NS, LAST_RESULTS
    args = (x, Wq, bq, Wk, bk, Wv, bv, Wp, bp, rope_cache)
    nc = _get_nc()
    in_maps = [make_in_map(c, *args) for c in range(NCORES)]
    r = None
    for attempt in range(4):
        try:
            r = run_bass_kernel_spmd(nc, in_maps, list(range(NCORES)))
            break
        except Exception:
            # transient NRT exec-unit errors recover on re-dispatch
            if attempt == 3:
                raise
            time.sleep(5.0 * (attempt + 1))
    LAST_EXEC_NS = r.exec_time_ns
    LAST_RESULTS = r
    out = np.zeros((2, T, C), np.float32)
    for core in range(NCORES):
        out[core // 4] += r.results[core]["out"].astype(np.float32)
    out += np.asarray(bp, np.float32)[None, None, :]
    return out


# revision 10
# speedup vs baseline: 1.4380x; 1.0190x over previous
"""Trainium2 Bass kernel for causal self-attention with RoPE.

Problem: B=2, T=2048, C=1024, H=16 heads, hd=64, fp32, causal, rotary embeddings.

Sharding: 8 cores = 2 batches x 4 head-groups. Core c handles batch c//4 and
heads [4*(c%4), 4*(c%4)+4). Each core computes its heads' Q/K/V projections,
RoPE, causal attention, and a partial output projection over its 256 input
channels; the host sums the 4 partial (fp16) projections per batch and adds
the output bias.

v3 design notes:
  - Attention tiled over FOUR 512-col t-windows, interleaved with projection
    work so ACT (exp, ~86us/core floor) starts early and the PE stays dense
    (HAM keeps the 2.4 GHz clock).
  - Both heads of a pair share one [128,1024] scores PSUM tile -> ONE wide
    ACTIVATE per s-tile covers both heads.
  - att@V + denominator matmuls pack the two heads into PE column groups;
    accumulation uses memset-zeroed PSUM with start=False (no accumulation
    groups), so the interleaved chains in one bank are legal.
  - In-loop emission lags att@V one s-tile behind exp so the PE FIFO never
    blocks on the ACT engine.
  - Inputs arrive in 14 large DMAs (descriptor issue on the sync queue costs
    ~0.6us each); rope's 32-row swap uses DVE copies, not DMA.
  - fp16 output partials, one wide DMA per 128-row t-chunk.
"""

import os
import time
from contextlib import ExitStack

import ml_dtypes
import numpy as np

import concourse.bass as bass
import concourse.tile as tile
from concourse import bacc, library_config, mybir
from concourse.bass_utils import run_bass_kernel_spmd

F32 = mybir.dt.float32
F16 = mybir.dt.float16
BF16 = mybir.dt.bfloat16

T = 2048
C = 1024
HD = 64
NCORES = 8
NEG = -1e10
NW = 4            # t-windows of 512
WW = 512          # window width

AF = mybir.ActivationFunctionType
ALU = mybir.AluOpType

LAST_EXEC_NS = None
LAST_RESULTS = None


def build_nc():
    nc = bacc.Bacc("TRN2", target_bir_lowering=False, debug=False)

    x3 = nc.dram_tensor("x3", [128, 8, T], BF16, kind="ExternalInput").ap()
    wq3 = nc.dram_tensor("wq3", [128, 8, 256], BF16, kind="ExternalInput").ap()
    wk3 = nc.dram_tensor("wk3", [128, 8, 256], BF16, kind="ExternalInput").ap()
    wv3 = nc.dram_tensor("wv3", [128, 9, 260], BF16, kind="ExternalInput").ap()
    wp3 = nc.dram_tensor("wp3", [128, 2, C], BF16, kind="ExternalInput").ap()
    bqk = nc.dram_tensor("bqk", [128, 4], F32, kind="ExternalInput").ap()
    cc_d = nc.dram_tensor("cc", [128, T], F32, kind="ExternalInput").ap()
    ss_d = nc.dram_tensor("ss", [128, T], F32, kind="ExternalInput").ap()
    tri_d = nc.dram_tensor("tri", [128, 128], F32, kind="ExternalInput").ap()
    out_d = nc.dram_tensor("out", [T, C], F16, kind="ExternalOutput").ap()

    with tile.TileContext(nc) as tc, ExitStack() as ctx:
        consts = ctx.enter_context(tc.tile_pool(name="consts", bufs=1))

        # ---- persistent SBUF tiles ----
        cc_sb = consts.tile([128, T], F32)
        ss_sb = consts.tile([128, T], F32)
        tri_sb = consts.tile([128, 128], F32)
        tri2 = consts.tile([128, 256], F32)
        tri2v = tri2.rearrange("p (h t) -> p h t", t=128)
        bqk_sb = consts.tile([128, 4], F32)
        x1 = consts.tile([1, T], BF16)          # ones row (V bias matmul)
        wsrc = consts.tile([128, 512], BF16)    # warmup matmul source

        xa = consts.tile([128, 8 * T], BF16, name="xa")
        xv = xa.rearrange("p (j t) -> p j t", t=T)
        wqa = consts.tile([128, 8 * 256], BF16, name="wqa")
        wqv = wqa.rearrange("p (j c) -> p j c", c=256)
        wka = consts.tile([128, 8 * 256], BF16, name="wka")
        wkv = wka.rearrange("p (j c) -> p j c", c=256)
        wva = consts.tile([128, 9 * 260], BF16, name="wva")
        wvv = wva.rearrange("p (j c) -> p j c", c=260)
        wpa = consts.tile([128, 2 * C], BF16, name="wpa")
        wpv = wpa.rearrange("p (q c) -> p q c", c=C)

        # rotated Q^T / K^T, one [128, T] tile per head pair:
        # 0,1 = Q pair0/1; 2,3 = K pair0/1. rows [hA(64) | hB(64)].
        qkt = [consts.tile([128, T], BF16, name=f"qkt{i}") for i in range(4)]
        # V' per s-tile: [128 s, 256] = 4 heads x 64 dims
        vp = [consts.tile([128, 260], BF16, name=f"vp{i}") for i in range(16)]
        # scaled attention outputs (d-major), one [128, T] per pair
        usc = [consts.tile([128, T], BF16, name=f"usc{p}") for p in range(2)]

        # ---- PSUM pools (8 banks total) ----
        sp = ctx.enter_context(
            tc.tile_pool(name="sp", bufs=2, space="PSUM"))      # 4 banks
        yp = ctx.enter_context(
            tc.tile_pool(name="yp", bufs=3, space="PSUM"))      # 3 banks
        qp = ctx.enter_context(
            tc.tile_pool(name="qp", bufs=1, space="PSUM"))      # 1 bank

        # ---- SBUF working pools ----
        etp = ctx.enter_context(tc.tile_pool(name="etp", bufs=8))
        rp = ctx.enter_context(tc.tile_pool(name="rp", bufs=2))
        dvp = ctx.enter_context(tc.tile_pool(name="dvp", bufs=2))
        ost = ctx.enter_context(tc.tile_pool(name="ost", bufs=4))

        # ================= t=0: warmup + setup =================
        nc.vector.memset(wsrc[:], 0.0)
        warm = sp.tile([128, 1024], F32, tag="s", name="warm")
        for r in range(14):
            nc.tensor.matmul(warm[:, 0:512], wsrc[:, 0:128], wsrc[:],
                             start=True, stop=True)
        nc.vector.memset(x1[:], 1.0)
        # preload the exp ACT table set early (table load ~2.7us)
        dummy_et = consts.tile([1, 128], BF16, name="dummy_et")
        nc.scalar.activation(dummy_et[:], wsrc[0:1, 0:128], AF.Exp, scale=0.125)
        nc.gpsimd.load_library(library_config.attn)

        # ================= DMA schedule (priority = order) =================
        def wsl(w):
            return slice(WW * w, WW * (w + 1))

        nc.sync.dma_start(wqa[:], wq3[:, :, :])
        nc.sync.dma_start(xv[:, :, wsl(0)], x3[:, :, wsl(0)])
        nc.sync.dma_start(bqk_sb[:], bqk[:])
        nc.sync.dma_start(cc_sb[:, 0:1024], cc_d[:, 0:1024])
        nc.sync.dma_start(ss_sb[:, 0:1024], ss_d[:, 0:1024])
        nc.sync.dma_start(wka[:], wk3[:, :, :])
        nc.sync.dma_start(tri_sb[:], tri_d[:])
        nc.sync.dma_start(tri2[:, 0:128], tri_d[:])
        nc.sync.dma_start(tri2[:, 128:256], tri_d[:])
        nc.sync.dma_start(wva[:], wv3[:, :, :])
        nc.sync.dma_start(xv[:, :, wsl(1)], x3[:, :, wsl(1)])
        nc.sync.dma_start(xv[:, :, wsl(2)], x3[:, :, wsl(2)])
        nc.sync.dma_start(cc_sb[:, 1024:2048], cc_d[:, 1024:2048])
        nc.sync.dma_start(ss_sb[:, 1024:2048], ss_d[:, 1024:2048])
        nc.sync.dma_start(xv[:, :, wsl(3)], x3[:, :, wsl(3)])
        nc.sync.dma_start(wpa[:], wp3[:, :, :])

        # ================= building blocks =================
        def qk_chunk(t_idx, w):
            """Project + rope one 512-col block of qkt[t_idx]."""
            wv_ = wqv if t_idx < 2 else wkv
            pr = t_idx % 2
            bcol = (2 if t_idx >= 2 else 0) + pr
            bias = bqk_sb[:, bcol:bcol + 1]
            ps = qp.tile([128, WW], F32, tag="qp", name=f"qk{t_idx}_{w}")
            for j in range(8):
                nc.tensor.matmul(
                    ps[:], wv_[:, j, 128 * pr:128 * (pr + 1)],
                    xv[:, j, wsl(w)], start=(j == 0), stop=(j == 7))
            qs = rp.tile([128, WW], F32, tag="qs")
            nc.vector.tensor_copy(qs[:], ps[:])   # frees the qp bank fast
            p1 = rp.tile([128, WW], F32, tag="p1")
            p2 = rp.tile([128, WW], F32, tag="p2")
            p2s = rp.tile([128, WW], F32, tag="p2s")
            nc.vector.scalar_tensor_tensor(
                out=p1[:], in0=qs[:], scalar=bias,
                in1=cc_sb[:, wsl(w)], op0=ALU.add, op1=ALU.mult)
            nc.vector.scalar_tensor_tensor(
                out=p2[:], in0=qs[:], scalar=bias,
                in1=ss_sb[:, wsl(w)], op0=ALU.add, op1=ALU.mult)
            for r in range(4):
                src = slice(32 * (r ^ 1), 32 * (r ^ 1) + 32)
                dst = slice(32 * r, 32 * r + 32)
                nc.sync.dma_start(p2s[dst, :], p2[src, :])
            nc.vector.tensor_add(qkt[t_idx][:, wsl(w)], p1[:], p2s[:])

        def v_chunk(tch):
            """V' for s-tile tch: [128 s, 256] = 4 heads x 64 dims."""
            vr = qp.tile([128, 260], F32, tag="qp", name=f"vr{tch}")
            tsl = slice(128 * tch, 128 * (tch + 1))
            for j in range(8):
                nc.tensor.matmul(vr[:], xv[:, j, tsl], wvv[:, j, :],
                                 start=(j == 0), stop=False)
            nc.tensor.matmul(vr[:], x1[:, tsl], wvv[0:1, 8, :],
                             start=False, stop=True)
            nc.vector.tensor_copy(vp[tch][:], vr[:])

        def attn_unit(p, w, u):
            """Attention for head-pair p over t-window w (s-tiles 0..4w+3)."""
            qt, kt = qkt[p], qkt[2 + p]
            ni = 4 * w + 4
            # per-head [65, WW] accumulators: rows 0:64 = y, row 64 = z
            # (ones column in V'), one PSUM bank each.
            yz = [yp.tile([65, WW], F32, tag="y", name=f"yz{p}_{w}_{hs}")
                  for hs in range(2)]
            ets = {}

            def scores_exp(i):
                lo = max(0, 128 * i - WW * w)
                sc = sp.tile([128, 1024], F32, tag="s", name=f"s{p}_{w}_{i}")
                # scores: A (rows 0:64) into cols lo:512, B (64:128) into
                # cols 512:1024 (untrimmed so the exp region is contiguous)
                nc.tensor.matmul(
                    sc[:, lo:WW],
                    kt[0:64, 128 * i:128 * (i + 1)],
                    qt[0:64, WW * w + lo:WW * (w + 1)],
                    start=True, stop=True)
                nc.tensor.matmul(
                    sc[:, WW:2 * WW],
                    kt[64:128, 128 * i:128 * (i + 1)],
                    qt[64:128, wsl(w)],
                    start=True, stop=True)
                if i >= 4 * w:  # diagonal tile: causal mask both heads
                    sc3 = sc.rearrange("p (h t) -> p h t", t=WW)
                    nc.vector.tensor_add(sc3[:, :, lo:lo + 128],
                                         sc3[:, :, lo:lo + 128], tri2v[:])
                et = etp.tile([128, 1024], BF16, tag="e", name=f"e{p}_{w}_{i}")
                nc.scalar.activation(et[:, lo:], sc[:, lo:], AF.Exp,
                                     scale=0.125)
                ets[i] = (et, lo)

            def yz_mm(i):
                et, lo = ets.pop(i)
                st, sp_ = (i == 0), (i == ni - 1)
                nc.tensor.matmul(yz[0][:, lo:],
                                 vp[i][:, 130 * p:130 * p + 65],
                                 et[:, lo:WW], start=st, stop=sp_)
                nc.tensor.matmul(yz[1][:, lo:],
                                 vp[i][:, 130 * p + 65:130 * p + 130],
                                 et[:, WW + lo:2 * WW], start=st, stop=sp_)

            # software pipeline: att@V lags exp by one s-tile so the PE FIFO
            # always has ready work while ACT computes the current exp.
            for i in range(ni):
                scores_exp(i)
                if i > 2:
                    yz_mm(i - 3)
            for i in range(max(0, ni - 3), ni):
                yz_mm(i)

            # normalize: usc[p][64hs:, w] = y * (1/z) broadcast per head
            for hs in range(2):
                zrow = dvp.tile([1, WW], F32, tag="zrow", name=f"zr{p}_{w}{hs}")
                rzr = dvp.tile([1, WW], F32, tag="rzr", name=f"rz{p}_{w}{hs}")
                rzb = dvp.tile([64, WW], F32, tag="rzb", name=f"rzb{p}_{w}{hs}")
                nc.vector.tensor_copy(zrow[:], yz[hs][64:65, :])
                nc.vector.reciprocal_approx_fast(rzr[:], zrow[:])
                nc.gpsimd.partition_broadcast(rzb[:], rzr[:])
                nc.vector.tensor_mul(usc[p][64 * hs:64 * (hs + 1), wsl(w)],
                                     yz[hs][0:64, :], rzb[:])

        def oproj_chunk(c, wide):
            """Output projection for t-chunk c (128 t-cols)."""
            tsl = slice(128 * c, 128 * (c + 1))
            st = ost.tile([128, C], F16, tag="o", name=f"os{c}")
            if not wide:
                for cg in range(2):
                    csl = slice(512 * cg, 512 * (cg + 1))
                    ps = qp.tile([128, 512], F32, tag="qp", name=f"op{c}_{cg}")
                    for pq in range(2):
                        nc.tensor.matmul(ps[:], usc[pq][:, tsl],
                                         wpv[:, pq, csl],
                                         start=(pq == 0), stop=(pq == 1))
                    nc.vector.tensor_copy(st[:, csl], ps[:])
            else:
                ps = sp.tile([128, 1024], F32, tag="s", name=f"opw{c}")
                for cg in range(2):
                    csl = slice(512 * cg, 512 * (cg + 1))
                    for pq in range(2):
                        nc.tensor.matmul(ps[:, csl], usc[pq][:, tsl],
                                         wpv[:, pq, csl],
                                         start=(pq == 0), stop=(pq == 1))
                nc.vector.tensor_copy(st[:], ps[:])
            nc.sync.dma_start(out_d[tsl, :], st[:])

        # ================= emission schedule =================
        # window-0 projections up front; thereafter window w+1's
        # projections are emitted inside window w's attention section so the
        # rope chain completes while the PE/ACT crunch the previous window.
        u = 0
        qk_chunk(0, 0); qk_chunk(2, 0)
        for tch in range(0, 4):
            v_chunk(tch)
        qk_chunk(1, 0); qk_chunk(3, 0)
        for w in range(NW):
            attn_unit(0, w, u); u += 1
            if w + 1 < NW:
                qk_chunk(0, w + 1); qk_chunk(2, w + 1)
            attn_unit(1, w, u); u += 1
            if w + 1 < NW:
                qk_chunk(1, w + 1); qk_chunk(3, w + 1)
                for tch in range(4 * w + 4, 4 * w + 8):
                    v_chunk(tch)
            for c in range(4 * w, 4 * w + 4):
                oproj_chunk(c, wide=(w == NW - 1))

    nc.compile()
    return nc


_NC_CACHE = {}


def _get_nc():
    if "nc" not in _NC_CACHE:
        _NC_CACHE["nc"] = build_nc()
    return _NC_CACHE["nc"]


def make_in_map(core, x, Wq, bq, Wk, bk, Wv, bv, Wp, bp, rope_cache):
    b = core // 4
    hbase = (core % 4) * 4

    pj_np = ml_dtypes.bfloat16
    xTa = np.empty((1024, T), np.float32)
    xTa[:] = np.asarray(x[b], np.float32).T
    x3a = np.ascontiguousarray(
        xTa.reshape(8, 128, T).transpose(1, 0, 2)).astype(pj_np)

    # packed channel order for Q/K: per pair p, heads (hbase+2p, hbase+2p+1),
    # rows [hA_even(32) | hA_odd(32) | hB_even(32) | hB_odd(32)]
    perm = []
    for p in range(2):
        for hh in range(2):
            h = hbase + 2 * p + hh
            perm += [h * HD + 2 * m for m in range(32)]
            perm += [h * HD + 2 * m + 1 for m in range(32)]
    perm = np.asarray(perm)

    wqTa = np.ascontiguousarray(np.asarray(Wq, np.float32)[perm, :].T)
    wq3a = np.ascontiguousarray(
        wqTa.reshape(8, 128, 256).transpose(1, 0, 2)).astype(pj_np)
    wkTa = np.ascontiguousarray(np.asarray(Wk, np.float32)[perm, :].T)
    wk3a = np.ascontiguousarray(
        wkTa.reshape(8, 128, 256).transpose(1, 0, 2)).astype(pj_np)

    chs = np.arange(hbase * HD, hbase * HD + 256)
    wvTa = np.zeros((9 * 128, 260), np.float32)
    WvT = np.asarray(Wv, np.float32)[chs, :].T     # [1024, 256]
    bvc = np.asarray(bv, np.float32)[chs]
    for lh in range(4):
        wvTa[:C, 65 * lh:65 * lh + 64] = WvT[:, 64 * lh:64 * lh + 64]
        wvTa[C, 65 * lh:65 * lh + 64] = bvc[64 * lh:64 * lh + 64]
        wvTa[C, 65 * lh + 64] = 1.0                # ones column (z row)
    wv3a = np.ascontiguousarray(
        wvTa.reshape(9, 128, 260).transpose(1, 0, 2)).astype(pj_np)

    wpTa = np.ascontiguousarray(np.asarray(Wp, np.float32)[:, chs].T)
    wp3a = np.ascontiguousarray(
        wpTa.reshape(2, 128, C).transpose(1, 0, 2)).astype(pj_np)

    bqp = np.asarray(bq, np.float32)[perm].reshape(2, 128).T
    bkp = np.asarray(bk, np.float32)[perm].reshape(2, 128).T
    bqk_a = np.concatenate([bqp, bkp], axis=1)  # [128, 4]

    rc = np.asarray(rope_cache, np.float32)  # [T, 32, 2]
    r = np.arange(128)
    m = r % 32
    sign = np.where((r % 64) < 32, 1.0, -1.0).astype(np.float32)
    cc_a = np.ascontiguousarray(rc[:, m, 0].T)            # [128, T]
    ss_a = np.ascontiguousarray(rc[:, m, 1].T * sign[:, None])

    sl, tl = np.arange(128)[:, None], np.arange(128)[None, :]
    tri_a = np.where(tl >= sl, 0.0, NEG).astype(np.float32)

    return dict(x3=x3a, wq3=wq3a, wk3=wk3a, wv3=wv3a, wp3=wp3a,
                bqk=bqk_a, cc=cc_a, ss=ss_a, tri=tri_a)


def kernel(x, Wq, bq, Wk, bk, Wv, bv, Wp, bp, rope_cache):
    global LAST_EXEC_NS, LAST_RESULTS
    args = (x, Wq, bq, Wk, bk, Wv, bv, Wp, bp, rope_cache)
    nc = _get_nc()
    in_maps = [make_in_map(c, *args) for c in range(NCORES)]
    r = None
    for attempt in range(4):
        try:
            r = run_bass_kernel_spmd(nc, in_maps, list(range(NCORES)))
            break
        except Exception:
            # transient NRT exec-unit errors recover on re-dispatch
            if attempt == 3:
                raise
            time.sleep(5.0 * (attempt + 1))
    LAST_EXEC_NS = r.exec_time_ns
    LAST_RESULTS = r
    out = np.zeros((2, T, C), np.float32)
    for core in range(NCORES):
        out[core // 4] += r.results[core]["out"].astype(np.float32)
    out += np.asarray(bp, np.float32)[None, None, :]
    return out
